# revision 13
# baseline (speedup 1.0000x reference)
"""Trainium2 Bass kernel for nn_BroadBINLayer (grouped log-softmax embedding).

Math:
  Wg = W.reshape(G, GS, C); theta = softmax(Wg, axis=1); logW = log(theta+eps)
  out = softmax(x_onehot @ logW + bias, axis=-1)

Key identity used here: x_onehot has exactly one active row per group per
sample, so
  x @ logW = x @ W - K,   K[c] = sum_g log(sum_r exp(W[g, r, c]))
(eps=1e-12 is below fp32 ulp of theta ~ 0.01, so log(theta+eps) == log(theta)
bit-exactly in fp32). The dense matmul therefore runs on RAW W (tiny values,
std ~ 0.0135, bf16-safe; no overflow so exp needs no max-subtraction), and the
grouped log-softmax collapses to an exp + segmented column-sum + log + a
per-class correction K folded into the final row-softmax.

Sharding: data-parallel over batch (4096 -> 8 x 512); W/seg/bias replicated.
Each core computes K redundantly (no collectives needed).
"""

import sys

import numpy as np
import ml_dtypes

sys.path.insert(0, "/opt/trn_rl_repo")

BATCH = 4096
ROWS = 10000
ROWS_PAD = 10112  # 79 * 128
NK = ROWS_PAD // 128  # 79
C = 1000
CH = 500  # class half
G = 100
NCORES = 8
BPC = BATCH // NCORES  # 512 rows of batch per core

_BF16 = ml_dtypes.bfloat16

_cache: dict = {}


def _build_bass():
    import concourse.bass as bass
    import concourse.bacc as bacc
    import concourse.tile as tile
    from concourse import mybir

    f32 = mybir.dt.float32
    bf16 = mybir.dt.bfloat16
    X = mybir.AxisListType.X
    Exp = mybir.ActivationFunctionType.Exp
    Ln = mybir.ActivationFunctionType.Ln

    nc = bacc.Bacc()
    xt = nc.dram_tensor("xt", [ROWS_PAD, BPC], bf16, kind="ExternalInput")
    w = nc.dram_tensor("w", [2, ROWS_PAD, CH], bf16, kind="ExternalInput")
    seg = nc.dram_tensor("seg", [ROWS_PAD, G], bf16, kind="ExternalInput")
    biasd = nc.dram_tensor("bias", [1, C], f32, kind="ExternalInput")
    outd = nc.dram_tensor("out", [BPC, C], f32, kind="ExternalOutput")

    with tile.TileContext(nc) as tc:
        with (
            tc.tile_pool(name="xpool", bufs=NK) as xpool,
            tc.tile_pool(name="wpool", bufs=6) as wpool,
            tc.tile_pool(name="epool", bufs=4) as epool,
            tc.tile_pool(name="spool", bufs=6) as spool,
            tc.tile_pool(name="singles", bufs=1) as singles,
            tc.tile_pool(name="lsb", bufs=1) as lsb,
            tc.tile_pool(name="fin", bufs=2) as fin,
            tc.tile_pool(name="psumL", bufs=4, space="PSUM") as psumL,
            tc.tile_pool(name="psumS", bufs=1, space="PSUM") as psumS,
            tc.tile_pool(name="psumK", bufs=1, space="PSUM") as psumK,
            tc.tile_pool(name="psumR", bufs=1, space="PSUM") as psumR,
        ):
            ones_g = singles.tile([G, 1], f32)
            nc.vector.memset(ones_g, 1.0)
            ones_p = singles.tile([1, 128], f32)
            nc.vector.memset(ones_p, 1.0)
            biast = singles.tile([1, C], f32)
            nc.sync.dma_start(out=biast, in_=biasd[:, :])
            logS = singles.tile([G, C], f32)
            kb = singles.tile([1, C], f32)
            kbrep = [
                psumR.tile([128, CH], f32, tag=f"kbrep{h}", name=f"kbrep{h}")
                for h in range(2)
            ]
            logits = [
                lsb.tile([128, C], f32, tag=f"l{m}", name=f"logits{m}")
                for m in range(4)
            ]
            xts = []

            for half in range(2):
                c0 = half * CH
                psums = [
                    psumL.tile([128, CH], f32, name=f"psum{m}", tag="Lp")
                    for m in range(4)
                ]
                psumS_t = psumS.tile([G, CH], f32)
                for k in range(NK):
                    r0 = k * 128
                    if half == 0:
                        x_new = xpool.tile([128, BPC], bf16)
                        nc.sync.dma_start(out=x_new, in_=xt[r0 : r0 + 128, :])
                        xts.append(x_new)
                    x_t = xts[k]
                    w_t = wpool.tile([128, CH], bf16)
                    nc.sync.dma_start(out=w_t, in_=w[half, r0 : r0 + 128, :])
                    s_t = spool.tile([128, G], bf16)
                    nc.sync.dma_start(out=s_t, in_=seg[r0 : r0 + 128, :])
                    e_t = epool.tile([128, CH], bf16)
                    nc.scalar.activation(out=e_t, in_=w_t, func=Exp)
                    nc.tensor.matmul(
                        psumS_t, lhsT=s_t, rhs=e_t, start=(k == 0), stop=(k == NK - 1)
                    )
                    for m in range(4):
                        nc.tensor.matmul(
                            psums[m],
                            lhsT=x_t[:, m * 128 : (m + 1) * 128],
                            rhs=w_t,
                            start=(k == 0),
                            stop=(k == NK - 1),
                        )
                # grouped log-softmax correction for this class half
                nc.scalar.activation(out=logS[:, c0 : c0 + CH], in_=psumS_t, func=Ln)
                psumK_t = psumK.tile([1, CH], f32)
                nc.tensor.matmul(
                    psumK_t,
                    lhsT=ones_g,
                    rhs=logS[:, c0 : c0 + CH],
                    start=True,
                    stop=True,
                )
                # kb = K - bias (to be subtracted from logits)
                nc.vector.tensor_sub(
                    out=kb[:, c0 : c0 + CH], in0=psumK_t, in1=biast[:, c0 : c0 + CH]
                )
                # replicate kb across 128 partitions via a rank-1 matmul
                nc.tensor.matmul(
                    kbrep[half],
                    lhsT=ones_p,
                    rhs=kb[:, c0 : c0 + CH],
                    start=True,
                    stop=True,
                )
                # evict this half's logits to SBUF, freeing PSUM for next half
                for m in range(4):
                    nc.vector.tensor_copy(
                        out=logits[m][:, c0 : c0 + CH], in_=psums[m]
                    )

            # final: out = softmax(logits - kbrep, axis=-1)
            for m in range(4):
                t_m = logits[m]
                for h in range(2):
                    nc.vector.tensor_sub(
                        out=t_m[:, h * CH : (h + 1) * CH],
                        in0=t_m[:, h * CH : (h + 1) * CH],
                        in1=kbrep[h],
                    )
                nrm = fin.tile([128, 1], f32, tag="nrm")
                nc.vector.reduce_max(out=nrm, in_=t_m, axis=X, negate=True)
                e_m = fin.tile([128, C], f32, tag="em")
                ssum = fin.tile([128, 1], f32, tag="ssum")
                nc.scalar.activation(
                    out=e_m, in_=t_m, func=Exp, bias=nrm, scale=1.0, accum_out=ssum
                )
                rec = fin.tile([128, 1], f32, tag="rec")
                nc.vector.reciprocal(out=rec, in_=ssum)
                o_m = fin.tile([128, C], f32, tag="om")
                nc.vector.tensor_scalar_mul(out=o_m, in0=e_m, scalar1=rec)
                nc.sync.dma_start(out=outd[m * 128 : (m + 1) * 128, :], in_=o_m)

    nc.finalize()
    return nc


def _get_nc():
    if "nc" not in _cache:
        _cache["nc"] = _build_bass()
    return _cache["nc"]


def _prep_inputs(x_onehot: np.ndarray, W_logits: np.ndarray, bias: np.ndarray):
    """Host-side staging: cast/transpose/pad/shard. Returns per-core in_maps."""
    xb = np.ascontiguousarray(x_onehot.T.astype(_BF16))  # (10000, 4096)
    w2 = np.zeros((2, ROWS_PAD, CH), dtype=_BF16)
    wb = W_logits.astype(_BF16)
    w2[0, :ROWS] = wb[:, :CH]
    w2[1, :ROWS] = wb[:, CH:]
    segm = np.zeros((ROWS_PAD, G), dtype=_BF16)
    segm[np.arange(ROWS), np.arange(ROWS) // (ROWS // G)] = 1
    bias2 = np.ascontiguousarray(bias.astype(np.float32).reshape(1, C))

    in_maps = []
    for i in range(NCORES):
        xi = np.zeros((ROWS_PAD, BPC), dtype=_BF16)
        xi[:ROWS] = xb[:, i * BPC : (i + 1) * BPC]
        in_maps.append({"xt": xi, "w": w2, "seg": segm, "bias": bias2})
    return in_maps


def kernel(x_onehot: np.ndarray, W_logits: np.ndarray, bias: np.ndarray) -> np.ndarray:
    from concourse.bass_utils import run_bass_kernel_spmd

    nc = _get_nc()
    in_maps = _prep_inputs(x_onehot, W_logits, bias)
    res = run_bass_kernel_spmd(nc, in_maps, list(range(NCORES)))
    out = np.concatenate([res.results[i]["out"] for i in range(NCORES)], axis=0)
    return out.astype(np.float32)


# revision 18
# speedup vs baseline: 1.1081x; 1.1081x over previous
"""Trainium2 Bass kernel for nn_BroadBINLayer (grouped log-softmax embedding).

Math:
  Wg = W.reshape(G, GS, C); theta = softmax(Wg, axis=1); logW = log(theta+eps)
  out = softmax(x_onehot @ logW + bias, axis=-1)

Key identity used here: x_onehot has exactly one active row per group per
sample, so
  x @ logW = x @ W - K,   K[c] = sum_g log(sum_r exp(W[g, r, c]))
(eps=1e-12 is below fp32 ulp of theta ~ 0.01, so log(theta+eps) == log(theta)
bit-exactly in fp32). The dense matmul therefore runs on RAW W (tiny values,
std ~ 0.0135, bf16-safe; no overflow so exp needs no max-subtraction), and the
grouped log-softmax collapses to an exp + segmented column-sum + log + a
per-class correction K folded into the final row-softmax.

Sharding: data-parallel over batch (4096 -> 8 x 512); W/seg/bias replicated.
Each core computes K redundantly (no collectives needed).
"""

import sys

import numpy as np
import ml_dtypes

sys.path.insert(0, "/opt/trn_rl_repo")

BATCH = 4096
ROWS = 10000
ROWS_PAD = 10112  # 79 * 128
NK = ROWS_PAD // 128  # 79
C = 1000
CH = 500  # class half
G = 100
NCORES = 8
BPC = BATCH // NCORES  # 512 rows of batch per core

_BF16 = ml_dtypes.bfloat16

_cache: dict = {}


def _build_bass():
    import concourse.bass as bass
    import concourse.bacc as bacc
    import concourse.tile as tile
    from concourse import mybir

    f32 = mybir.dt.float32
    bf16 = mybir.dt.bfloat16
    X = mybir.AxisListType.X
    Exp = mybir.ActivationFunctionType.Exp
    Ln = mybir.ActivationFunctionType.Ln

    nc = bacc.Bacc()
    xt = nc.dram_tensor("xt", [ROWS_PAD, BPC], bf16, kind="ExternalInput")
    w = nc.dram_tensor("w", [2, ROWS_PAD, CH], bf16, kind="ExternalInput")
    seg = nc.dram_tensor("seg", [ROWS_PAD, G], bf16, kind="ExternalInput")
    biasd = nc.dram_tensor("bias", [1, C], f32, kind="ExternalInput")
    outd = nc.dram_tensor("out", [BPC, C], f32, kind="ExternalOutput")

    with tile.TileContext(nc) as tc:
        with (
            tc.tile_pool(name="xpool", bufs=NK) as xpool,
            tc.tile_pool(name="wpool", bufs=12) as wpool,
            tc.tile_pool(name="epool", bufs=8) as epool,
            tc.tile_pool(name="spool", bufs=NK) as spool,
            tc.tile_pool(name="singles", bufs=1) as singles,
            tc.tile_pool(name="lsb", bufs=1) as lsb,
            tc.tile_pool(name="fin", bufs=2) as fin,
            tc.tile_pool(name="psumL", bufs=4, space="PSUM") as psumL,
            tc.tile_pool(name="psumS", bufs=1, space="PSUM") as psumS,
            tc.tile_pool(name="psumK", bufs=1, space="PSUM") as psumK,
            tc.tile_pool(name="psumR", bufs=1, space="PSUM") as psumR,
        ):
            ones_g = singles.tile([G, 1], f32)
            nc.vector.memset(ones_g, 1.0)
            ones_p = singles.tile([1, 128], f32)
            nc.vector.memset(ones_p, 1.0)
            biast = singles.tile([1, C], f32)
            nc.sync.dma_start(out=biast, in_=biasd[:, :])
            logS = singles.tile([G, C], f32)
            kb = singles.tile([1, C], f32)
            kbrep = [
                psumR.tile([128, CH], f32, tag=f"kbrep{h}", name=f"kbrep{h}")
                for h in range(2)
            ]
            logits = [
                lsb.tile([128, C], f32, tag=f"l{m}", name=f"logits{m}")
                for m in range(4)
            ]
            xts = []
            segs = []

            for half in range(2):
                c0 = half * CH
                psums = [
                    psumL.tile([128, CH], f32, name=f"psum{m}", tag="Lp")
                    for m in range(4)
                ]
                psumS_t = psumS.tile([G, CH], f32)
                for k in range(NK):
                    r0 = k * 128
                    if half == 0:
                        x_new = xpool.tile([128, BPC], bf16)
                        nc.sync.dma_start(out=x_new, in_=xt[r0 : r0 + 128, :])
                        xts.append(x_new)
                        s_new = spool.tile([128, G], bf16)
                        nc.sync.dma_start(out=s_new, in_=seg[r0 : r0 + 128, :])
                        segs.append(s_new)
                    x_t = xts[k]
                    s_t = segs[k]
                    w_t = wpool.tile([128, CH], bf16)
                    nc.sync.dma_start(out=w_t, in_=w[half, r0 : r0 + 128, :])
                    e_t = epool.tile([128, CH], bf16)
                    nc.scalar.activation(out=e_t, in_=w_t, func=Exp)
                    nc.tensor.matmul(
                        psumS_t, lhsT=s_t, rhs=e_t, start=(k == 0), stop=(k == NK - 1)
                    )
                    for m in range(4):
                        nc.tensor.matmul(
                            psums[m],
                            lhsT=x_t[:, m * 128 : (m + 1) * 128],
                            rhs=w_t,
                            start=(k == 0),
                            stop=(k == NK - 1),
                        )
                # grouped log-softmax correction for this class half
                nc.scalar.activation(out=logS[:, c0 : c0 + CH], in_=psumS_t, func=Ln)
                psumK_t = psumK.tile([1, CH], f32)
                nc.tensor.matmul(
                    psumK_t,
                    lhsT=ones_g,
                    rhs=logS[:, c0 : c0 + CH],
                    start=True,
                    stop=True,
                )
                # kb = K - bias (to be subtracted from logits)
                nc.vector.tensor_sub(
                    out=kb[:, c0 : c0 + CH], in0=psumK_t, in1=biast[:, c0 : c0 + CH]
                )
                # replicate kb across 128 partitions via a rank-1 matmul
                nc.tensor.matmul(
                    kbrep[half],
                    lhsT=ones_p,
                    rhs=kb[:, c0 : c0 + CH],
                    start=True,
                    stop=True,
                )
                if half == 0:
                    # evict pass-A logits quickly (plain copy) so pass B can
                    # reuse the PSUM banks; subtract kb for this half during
                    # pass B (DVE is idle then).
                    for m in range(4):
                        nc.vector.tensor_copy(
                            out=logits[m][:, c0 : c0 + CH], in_=psums[m]
                        )
                    for m in range(4):
                        nc.vector.tensor_sub(
                            out=logits[m][:, c0 : c0 + CH],
                            in0=logits[m][:, c0 : c0 + CH],
                            in1=kbrep[0],
                        )
                else:
                    # fused evict+subtract for the last half (DVE can read
                    # only one PSUM operand, so stage kbrep in SBUF first)
                    kbrep1_sb = singles.tile([128, CH], f32)
                    nc.vector.tensor_copy(out=kbrep1_sb, in_=kbrep[1])
                    for m in range(4):
                        nc.vector.tensor_sub(
                            out=logits[m][:, c0 : c0 + CH],
                            in0=psums[m],
                            in1=kbrep1_sb,
                        )

            # softmax shift: s = mean_c(kb), identical on every partition of
            # kbrep, so a free-dim reduce gives it per-partition directly.
            # |logits - kb + s| is O(1), so exp needs no row-max subtraction.
            r1 = fin.tile([128, 1], f32, tag="r1", bufs=1)
            nc.vector.reduce_sum(out=r1, in_=kbrep[0], axis=X)
            r2 = fin.tile([128, 1], f32, tag="r2", bufs=1)
            nc.vector.reduce_sum(out=r2, in_=kbrep[1], axis=X)
            s_rep = fin.tile([128, 1], f32, tag="srep", bufs=1)
            nc.vector.tensor_add(out=s_rep, in0=r1, in1=r2)
            nc.scalar.mul(out=s_rep, in_=s_rep, mul=1.0 / C)

            # final: out = softmax(logits - kb) = e / sum(e), e = exp(t + s)
            for m in range(4):
                e_m = fin.tile([128, C], f32, tag="em")
                ssum = fin.tile([128, 1], f32, tag="ssum")
                nc.scalar.activation(
                    out=e_m,
                    in_=logits[m],
                    func=Exp,
                    bias=s_rep,
                    scale=1.0,
                    accum_out=ssum,
                )
                rec = fin.tile([128, 1], f32, tag="rec")
                nc.vector.reciprocal(out=rec, in_=ssum)
                o_m = fin.tile([128, C], f32, tag="om")
                nc.vector.tensor_scalar_mul(out=o_m, in0=e_m, scalar1=rec)
                nc.sync.dma_start(out=outd[m * 128 : (m + 1) * 128, :], in_=o_m)

    nc.finalize()
    return nc


def _get_nc():
    if "nc" not in _cache:
        _cache["nc"] = _build_bass()
    return _cache["nc"]


def _prep_inputs(x_onehot: np.ndarray, W_logits: np.ndarray, bias: np.ndarray):
    """Host-side staging: cast/transpose/pad/shard. Returns per-core in_maps."""
    xb = np.ascontiguousarray(x_onehot.T.astype(_BF16))  # (10000, 4096)
    w2 = np.zeros((2, ROWS_PAD, CH), dtype=_BF16)
    wb = W_logits.astype(_BF16)
    w2[0, :ROWS] = wb[:, :CH]
    w2[1, :ROWS] = wb[:, CH:]
    segm = np.zeros((ROWS_PAD, G), dtype=_BF16)
    segm[np.arange(ROWS), np.arange(ROWS) // (ROWS // G)] = 1
    bias2 = np.ascontiguousarray(bias.astype(np.float32).reshape(1, C))

    in_maps = []
    for i in range(NCORES):
        xi = np.zeros((ROWS_PAD, BPC), dtype=_BF16)
        xi[:ROWS] = xb[:, i * BPC : (i + 1) * BPC]
        in_maps.append({"xt": xi, "w": w2, "seg": segm, "bias": bias2})
    return in_maps


def kernel(x_onehot: np.ndarray, W_logits: np.ndarray, bias: np.ndarray) -> np.ndarray:
    from concourse.bass_utils import run_bass_kernel_spmd

    nc = _get_nc()
    in_maps = _prep_inputs(x_onehot, W_logits, bias)
    res = run_bass_kernel_spmd(nc, in_maps, list(range(NCORES)))
    out = np.concatenate([res.results[i]["out"] for i in range(NCORES)], axis=0)
    return out.astype(np.float32)


# revision 24
# speedup vs baseline: 1.3154x; 1.1871x over previous
"""Trainium2 Bass kernel for nn_BroadBINLayer (grouped log-softmax embedding).

Math:
  Wg = W.reshape(G, GS, C); theta = softmax(Wg, axis=1); logW = log(theta+eps)
  out = softmax(x_onehot @ logW + bias, axis=-1)

Key identity used here: x_onehot has exactly one active row per group per
sample, so
  x @ logW = x @ W - K,   K[c] = sum_g log(sum_r exp(W[g, r, c]))
(eps=1e-12 is below fp32 ulp of theta ~ 0.01, so log(theta+eps) == log(theta)
bit-exactly in fp32). The dense matmul therefore runs on RAW W (tiny values,
std ~ 0.0135, bf16-safe; no overflow so exp needs no max-subtraction), and the
grouped log-softmax collapses to an exp + segmented column-sum + log + a
per-class correction K folded into the final row-softmax.

Sharding: data-parallel over batch (4096 -> 8 x 512); W/seg/bias replicated.
Each core computes K redundantly (no collectives needed).
"""

import sys

import numpy as np
import ml_dtypes

sys.path.insert(0, "/opt/trn_rl_repo")

BATCH = 4096
ROWS = 10000
ROWS_PAD = 10112  # 79 * 128
NK = ROWS_PAD // 128  # 79
C = 1000
CH = 500  # class half
G = 100
NCORES = 8
BPC = BATCH // NCORES  # 512 rows of batch per core

_BF16 = ml_dtypes.bfloat16

_cache: dict = {}


def _build_bass():
    import concourse.bass as bass
    import concourse.bacc as bacc
    import concourse.tile as tile
    from concourse import mybir

    f32 = mybir.dt.float32
    bf16 = mybir.dt.bfloat16
    X = mybir.AxisListType.X
    Exp = mybir.ActivationFunctionType.Exp
    Ln = mybir.ActivationFunctionType.Ln

    nc = bacc.Bacc()
    # xs packs the transposed one-hot shard [:, :512] and the group-membership
    # matrix [:, 512:612] so each k-tile arrives in a single DMA.
    xs = nc.dram_tensor("xs", [ROWS_PAD, BPC + G], bf16, kind="ExternalInput")
    w = nc.dram_tensor("w", [2, ROWS_PAD, CH], bf16, kind="ExternalInput")
    biasd = nc.dram_tensor("bias", [1, C], f32, kind="ExternalInput")
    outd = nc.dram_tensor("out", [BPC, C], f32, kind="ExternalOutput")

    with tile.TileContext(nc) as tc:
        with (
            tc.tile_pool(name="xpool", bufs=NK) as xpool,
            tc.tile_pool(name="wpool", bufs=16) as wpool,
            tc.tile_pool(name="epool", bufs=10) as epool,
            tc.tile_pool(name="singles", bufs=1) as singles,
            tc.tile_pool(name="lsb", bufs=1) as lsb,
            tc.tile_pool(name="fin", bufs=2) as fin,
            tc.tile_pool(name="psumL", bufs=4, space="PSUM") as psumL,
            tc.tile_pool(name="psumS", bufs=1, space="PSUM") as psumS,
            tc.tile_pool(name="psumK", bufs=1, space="PSUM") as psumK,
            tc.tile_pool(name="psumR", bufs=1, space="PSUM") as psumR,
        ):
            ones_g = singles.tile([G, 1], f32)
            nc.vector.memset(ones_g, 1.0)
            ones_p = singles.tile([1, 128], f32)
            nc.vector.memset(ones_p, 1.0)
            biast = singles.tile([1, C], f32)
            nc.sync.dma_start(out=biast, in_=biasd[:, :])
            logS = singles.tile([G, C], f32)
            kb = singles.tile([1, C], f32)
            kbrep = [
                psumR.tile([128, CH], f32, tag=f"kbrep{h}", name=f"kbrep{h}")
                for h in range(2)
            ]
            logits = [
                lsb.tile([128, C], f32, tag=f"l{m}", name=f"logits{m}")
                for m in range(4)
            ]
            xts = []

            for half in range(2):
                c0 = half * CH
                psums = [
                    psumL.tile([128, CH], f32, name=f"psum{m}", tag="Lp")
                    for m in range(4)
                ]
                psumS_t = psumS.tile([G, CH], f32)
                for k in range(NK):
                    r0 = k * 128
                    if half == 0:
                        x_new = xpool.tile([128, BPC + G], bf16)
                        nc.sync.dma_start(out=x_new, in_=xs[r0 : r0 + 128, :])
                        xts.append(x_new)
                    x_t = xts[k]
                    s_t = x_t[:, BPC : BPC + G]
                    w_t = wpool.tile([128, CH], bf16)
                    nc.sync.dma_start(out=w_t, in_=w[half, r0 : r0 + 128, :])
                    e_t = epool.tile([128, CH], bf16)
                    nc.scalar.activation(out=e_t, in_=w_t, func=Exp)
                    nc.tensor.matmul(
                        psumS_t, lhsT=s_t, rhs=e_t, start=(k == 0), stop=(k == NK - 1)
                    )
                    for m in range(4):
                        nc.tensor.matmul(
                            psums[m],
                            lhsT=x_t[:, m * 128 : (m + 1) * 128],
                            rhs=w_t,
                            start=(k == 0),
                            stop=(k == NK - 1),
                        )
                # grouped log-softmax correction for this class half
                nc.scalar.activation(out=logS[:, c0 : c0 + CH], in_=psumS_t, func=Ln)
                psumK_t = psumK.tile([1, CH], f32)
                nc.tensor.matmul(
                    psumK_t,
                    lhsT=ones_g,
                    rhs=logS[:, c0 : c0 + CH],
                    start=True,
                    stop=True,
                )
                # kb = K - bias (to be subtracted from logits)
                nc.vector.tensor_sub(
                    out=kb[:, c0 : c0 + CH], in0=psumK_t, in1=biast[:, c0 : c0 + CH]
                )
                # replicate kb across 128 partitions via a rank-1 matmul
                nc.tensor.matmul(
                    kbrep[half],
                    lhsT=ones_p,
                    rhs=kb[:, c0 : c0 + CH],
                    start=True,
                    stop=True,
                )
                if half == 0:
                    # evict pass-A logits quickly (plain copy) so pass B can
                    # reuse the PSUM banks; subtract kb for this half during
                    # pass B (DVE is idle then).
                    for m in range(4):
                        nc.vector.tensor_copy(
                            out=logits[m][:, c0 : c0 + CH], in_=psums[m]
                        )
                    for m in range(4):
                        nc.vector.tensor_sub(
                            out=logits[m][:, c0 : c0 + CH],
                            in0=logits[m][:, c0 : c0 + CH],
                            in1=kbrep[0],
                        )
                else:
                    # softmax shift: s = mean_c(kb), identical on every
                    # partition of kbrep, so free-dim reduces give it
                    # per-partition directly. |logits - kb + s| is O(1), so
                    # exp needs no row-max subtraction.
                    r1 = fin.tile([128, 1], f32, tag="r1", bufs=1)
                    nc.vector.reduce_sum(out=r1, in_=kbrep[0], axis=X)
                    r2 = fin.tile([128, 1], f32, tag="r2", bufs=1)
                    nc.vector.reduce_sum(out=r2, in_=kbrep[1], axis=X)
                    s_rep = fin.tile([128, 1], f32, tag="srep", bufs=1)
                    nc.vector.tensor_add(out=s_rep, in0=r1, in1=r2)
                    nc.scalar.mul(out=s_rep, in_=s_rep, mul=1.0 / C)
                    # fused evict+subtract for the last half (DVE can read
                    # only one PSUM operand, so stage kbrep in SBUF first),
                    # interleaved per-m with the softmax so ACT starts early
                    kbrep1_sb = singles.tile([128, CH], f32)
                    nc.vector.tensor_copy(out=kbrep1_sb, in_=kbrep[1])
                    for m in range(4):
                        nc.vector.tensor_sub(
                            out=logits[m][:, c0 : c0 + CH],
                            in0=psums[m],
                            in1=kbrep1_sb,
                        )
                        e_m = fin.tile([128, C], f32, tag="em")
                        ssum = fin.tile([128, 1], f32, tag="ssum")
                        nc.scalar.activation(
                            out=e_m,
                            in_=logits[m],
                            func=Exp,
                            bias=s_rep,
                            scale=1.0,
                            accum_out=ssum,
                        )
                        rec = fin.tile([128, 1], f32, tag="rec")
                        nc.vector.reciprocal(out=rec, in_=ssum)
                        o_m = fin.tile([128, C], f32, tag="om")
                        nc.vector.tensor_scalar_mul(out=o_m, in0=e_m, scalar1=rec)
                        nc.sync.dma_start(
                            out=outd[m * 128 : (m + 1) * 128, :], in_=o_m
                        )

    nc.finalize()
    return nc


def _get_nc():
    if "nc" not in _cache:
        _cache["nc"] = _build_bass()
    return _cache["nc"]


def _prep_inputs(x_onehot: np.ndarray, W_logits: np.ndarray, bias: np.ndarray):
    """Host-side staging: cast/transpose/pad/shard. Returns per-core in_maps."""
    xb = np.ascontiguousarray(x_onehot.T.astype(_BF16))  # (10000, 4096)
    w2 = np.zeros((2, ROWS_PAD, CH), dtype=_BF16)
    wb = W_logits.astype(_BF16)
    w2[0, :ROWS] = wb[:, :CH]
    w2[1, :ROWS] = wb[:, CH:]
    segm = np.zeros((ROWS_PAD, G), dtype=_BF16)
    segm[np.arange(ROWS), np.arange(ROWS) // (ROWS // G)] = 1
    bias2 = np.ascontiguousarray(bias.astype(np.float32).reshape(1, C))

    in_maps = []
    for i in range(NCORES):
        xi = np.zeros((ROWS_PAD, BPC + G), dtype=_BF16)
        xi[:ROWS, :BPC] = xb[:, i * BPC : (i + 1) * BPC]
        xi[:, BPC:] = segm
        in_maps.append({"xs": xi, "w": w2, "bias": bias2})
    return in_maps


def kernel(x_onehot: np.ndarray, W_logits: np.ndarray, bias: np.ndarray) -> np.ndarray:
    from concourse.bass_utils import run_bass_kernel_spmd

    nc = _get_nc()
    in_maps = _prep_inputs(x_onehot, W_logits, bias)
    res = run_bass_kernel_spmd(nc, in_maps, list(range(NCORES)))
    out = np.concatenate([res.results[i]["out"] for i in range(NCORES)], axis=0)
    return out.astype(np.float32)
